# revision 5
# baseline (speedup 1.0000x reference)
"""MoE top-2 expert projection kernel for 8 Trainium2 NeuronCores.

Computation (matches the reference nn.Module):
    gate_logits = x @ Wg.T + bg            [B,S,E]
    scores      = softmax(gate_logits)     over E=8
    top2        = top_k(scores, 2)
    out         = sum_k scores_k * (x @ W_{idx_k}.T + b_{idx_k})

Strategy: data-parallel over tokens (8192 tokens -> 1024/core), dense
compute over all 8 experts with the non-top-2 contributions zeroed via the
gate-weight matrix w[t,e] (w = scores * top2_mask).  On each core:
  - gate logits computed in full fp32 on the PE (top-2 selection is
    numerically sensitive: min rank2/3 logit gap in this regime ~5e-5)
  - expert projections in fp16 (x, W cast on the fly by SWDGE DMA),
    accumulated in fp32 PSUM over the D=1024 contraction
  - per-token combine on the VectorE: acc = psum_y * w[:,e] + acc
  - bias folded in as a single small matmul: acc_init = w @ b (K=8)

The expert loop is outermost so expert e's matmuls overlap expert e+1's
weight DMA (32 MiB of weights total vs ~27 us of matmul per expert).

Host side only reshapes/transposes/shards numpy arrays; all arithmetic
runs on the NeuronCores.
"""

import sys

if "/opt/trn_rl_repo" not in sys.path:
    sys.path.insert(0, "/opt/trn_rl_repo")

import numpy as np

import concourse.bass as bass
import concourse.mybir as mybir
import concourse.tile as tile
from concourse import bacc
from concourse.bass import ts
from concourse.bass_utils import run_bass_kernel_spmd
from concourse.masks import make_identity

# Problem sizes (hardcoded per the harness contract).
B, S, D, O, E = 4, 2048, 1024, 1024, 8
N_CORES = 8
TOK = B * S                  # 8192 tokens total
TPC = TOK // N_CORES         # 1024 tokens per core
P = 128                      # SBUF partitions
KO = D // P                  # 8 contraction blocks
TT = TPC // P                # 8 token tiles per core
NH = O // 512                # 2 PSUM halves of the output dim

CDT = mybir.dt.float16       # compute dtype for the expert matmuls
F32 = mybir.dt.float32


def build_nc(y_bufs: int = 2, with_debug: bool = False):
    nc = bacc.Bacc(None, target_bir_lowering=False)

    xT = nc.dram_tensor("xT", [D, TPC], F32, kind="ExternalInput")
    WT = nc.dram_tensor("WT", [E, D, O], F32, kind="ExternalInput")
    WgT = nc.dram_tensor("WgT", [D, E], F32, kind="ExternalInput")
    b_in = nc.dram_tensor("b", [E, O], F32, kind="ExternalInput")
    bg_in = nc.dram_tensor("bg", [1, E], F32, kind="ExternalInput")
    out = nc.dram_tensor("out", [TPC, O], F32, kind="ExternalOutput")
    if with_debug:
        g_dbg = nc.dram_tensor("g_dbg", [TPC, E], F32, kind="ExternalOutput")
        w_dbg = nc.dram_tensor("w_dbg", [TPC, E], F32, kind="ExternalOutput")

    from contextlib import ExitStack

    with tile.TileContext(nc) as tc, ExitStack() as stack:
        rpool = stack.enter_context(tc.tile_pool(name="resident", bufs=1))
        wpool = stack.enter_context(tc.tile_pool(name="work", bufs=3))
        ypool = stack.enter_context(
            tc.tile_pool(name="psum_y", bufs=y_bufs, space="PSUM")
        )
        spool = stack.enter_context(tc.tile_pool(name="psum_s", bufs=2, space="PSUM"))
        gstack = stack.enter_context(ExitStack())
        gpool = gstack.enter_context(tc.tile_pool(name="gwork", bufs=1))
        if True:
            # ---- input loads (program order = DMA priority order) ---------
            # gate path inputs first so gating can start immediately
            wg32 = rpool.tile([P, KO, E], F32, tag="wg32")
            nc.sync.dma_start(
                wg32[:], WgT[:, :].rearrange("(ko p) e -> p ko e", p=P)
            )
            bg32 = rpool.tile([1, E], F32, tag="bg32")
            nc.sync.dma_start(bg32[:], bg_in[:, :])
            # xT32 is only read during gating; its SBUF space is released
            # afterwards (tile lifetime), making room for the accumulators.
            xT32 = gpool.tile([P, KO, TPC], F32, tag="xT32")
            for ko in range(KO):
                nc.sync.dma_start(xT32[:, ko, :], xT[ko * P:(ko + 1) * P, :])
            xT16 = rpool.tile([P, KO, TPC], CDT, tag="xT16")
            for ko in range(KO):
                nc.gpsimd.dma_start(xT16[:, ko, :], xT[ko * P:(ko + 1) * P, :])
            b16 = rpool.tile([E, O], CDT, tag="b16")
            nc.gpsimd.dma_start(b16[:], b_in[:, :])

            wt16 = []
            for e in range(E):
                wte = rpool.tile([P, KO, O], CDT, tag=f"wt16_{e}")
                for ko in range(KO):
                    nc.gpsimd.dma_start(
                        wte[:, ko, :], WT[e, ko * P:(ko + 1) * P, :]
                    )
                wt16.append(wte)

            ones32 = rpool.tile([1, P], F32, tag="ones32")
            nc.gpsimd.memset(ones32[:], 1.0)
            ident = rpool.tile([P, P], F32, tag="ident")
            make_identity(nc, ident[:])

            # ---- phase 1: gating for all token tiles ----------------------
            w_tiles = []
            wt_tiles = []
            acc_tiles = []
            for tt in range(TT):
                tsl = ts(tt, P)

                # gate logits in fp32: psum_g = xT.T @ WgT + 1.T @ bg
                psum_g = spool.tile([P, E], F32, tag="small")
                for ko in range(KO):
                    nc.tensor.matmul(
                        psum_g[:],
                        lhsT=xT32[:, ko, tsl],
                        rhs=wg32[:, ko, :],
                        start=(ko == 0),
                        stop=False,
                    )
                nc.tensor.matmul(
                    psum_g[:], lhsT=ones32[:], rhs=bg32[:], start=False, stop=True
                )

                logits = wpool.tile([P, E], F32, tag="logits")
                nc.any.tensor_copy(logits[:], psum_g[:])

                # top-2 mask + softmax weights, all per-token (per-partition)
                m1 = wpool.tile([P, 1], F32, tag="m1")
                nc.vector.tensor_reduce(
                    m1[:], logits[:], mybir.AxisListType.X, mybir.AluOpType.max
                )
                negm1 = wpool.tile([P, 1], F32, tag="negm1")
                nc.vector.tensor_scalar_mul(negm1[:], m1[:], -1.0)
                eq1 = wpool.tile([P, E], F32, tag="eq1")
                nc.vector.tensor_scalar(
                    eq1[:], logits[:], m1[:], None, mybir.AluOpType.is_equal
                )
                masked = wpool.tile([P, E], F32, tag="masked")
                # masked = logits - 1e30 * eq1
                nc.vector.scalar_tensor_tensor(
                    masked[:], eq1[:], -1e30, logits[:],
                    mybir.AluOpType.mult, mybir.AluOpType.add,
                )
                m2 = wpool.tile([P, 1], F32, tag="m2")
                nc.vector.tensor_reduce(
                    m2[:], masked[:], mybir.AxisListType.X, mybir.AluOpType.max
                )
                eq2 = wpool.tile([P, E], F32, tag="eq2")
                nc.vector.tensor_scalar(
                    eq2[:], masked[:], m2[:], None, mybir.AluOpType.is_equal
                )
                wmask = wpool.tile([P, E], F32, tag="wmask")
                nc.vector.tensor_tensor(
                    wmask[:], eq1[:], eq2[:], mybir.AluOpType.add
                )

                ex = wpool.tile([P, E], F32, tag="ex")
                nc.scalar.activation(
                    ex[:], logits[:], mybir.ActivationFunctionType.Exp,
                    bias=negm1[:, 0:1], scale=1.0,
                )
                ssum = wpool.tile([P, 1], F32, tag="ssum")
                nc.vector.tensor_reduce(
                    ssum[:], ex[:], mybir.AxisListType.X, mybir.AluOpType.add
                )
                rsum = wpool.tile([P, 1], F32, tag="rsum")
                nc.vector.reciprocal(rsum[:], ssum[:])
                w = rpool.tile([P, E], F32, tag=f"w_{tt}")
                # w = (ex * rsum) * wmask  -> scores with non-top2 zeroed
                nc.vector.tensor_scalar(
                    w[:], ex[:], rsum[:], None, mybir.AluOpType.mult
                )
                nc.vector.tensor_tensor(w[:], w[:], wmask[:], mybir.AluOpType.mult)
                w_tiles.append(w)

                if with_debug:
                    nc.sync.dma_start(g_dbg[tsl, :], logits[:])
                    nc.sync.dma_start(w_dbg[tsl, :], w[:])

                # wT (fp16) for the bias matmul: [E, P]
                psum_wt = spool.tile([E, P], F32, tag="small")
                nc.tensor.transpose(psum_wt[:], w[:], ident[:])
                wt = rpool.tile([E, P], CDT, tag=f"wtT_{tt}")
                nc.any.tensor_copy(wt[:], psum_wt[:])
                wt_tiles.append(wt)

            # xT32 is dead now; release its SBUF for the accumulators.
            gstack.close()
            apool = stack.enter_context(tc.tile_pool(name="accs", bufs=1))

            # ---- phase 1b: acc starts as the weighted bias: acc = w @ b ---
            for tt in range(TT):
                psum_b = ypool.tile([P, O], F32, tag="y")
                for h in range(NH):
                    hsl = ts(h, 512)
                    nc.tensor.matmul(
                        psum_b[:, hsl], lhsT=wt_tiles[tt][:], rhs=b16[:, hsl],
                        start=True, stop=True,
                    )
                acc = apool.tile([P, O], F32, tag=f"acc_{tt}")
                nc.any.tensor_copy(acc[:], psum_b[:])
                acc_tiles.append(acc)

            # ---- phase 2: expert-major dense matmuls ----------------------
            for e in range(E):
                for tt in range(TT):
                    tsl = ts(tt, P)
                    psum_y = ypool.tile([P, O], F32, tag="y")
                    for ko in range(KO):
                        for h in range(NH):
                            hsl = ts(h, 512)
                            nc.tensor.matmul(
                                psum_y[:, hsl],
                                lhsT=xT16[:, ko, tsl],
                                rhs=wt16[e][:, ko, hsl],
                                start=(ko == 0),
                                stop=(ko == KO - 1),
                            )
                    # acc += psum_y * w[:, e]
                    nc.vector.scalar_tensor_tensor(
                        acc_tiles[tt][:], psum_y[:], w_tiles[tt][:, e:e + 1],
                        acc_tiles[tt][:],
                        mybir.AluOpType.mult, mybir.AluOpType.add,
                    )

            # ---- phase 3: store -------------------------------------------
            for tt in range(TT):
                nc.sync.dma_start(out[ts(tt, P), :], acc_tiles[tt][:])

    nc.compile()
    return nc


_NC_CACHE = {}


def _get_nc(with_debug: bool = False):
    key = with_debug
    if key not in _NC_CACHE:
        _NC_CACHE[key] = build_nc(with_debug=with_debug)
    return _NC_CACHE[key]


def _make_in_maps(x, W, b, Wg, bg):
    x = np.ascontiguousarray(x, dtype=np.float32)
    xf = x.reshape(TOK, D)
    WT = np.ascontiguousarray(np.asarray(W, np.float32).transpose(0, 2, 1))
    WgT = np.ascontiguousarray(np.asarray(Wg, np.float32).T)
    b = np.ascontiguousarray(b, dtype=np.float32)
    bg2 = np.ascontiguousarray(np.asarray(bg, np.float32).reshape(1, E))
    in_maps = []
    for c in range(N_CORES):
        xTc = np.ascontiguousarray(xf[c * TPC:(c + 1) * TPC].T)
        in_maps.append({"xT": xTc, "WT": WT, "WgT": WgT, "b": b, "bg": bg2})
    return in_maps


def run(inputs, with_debug=False, **spmd_kwargs):
    nc = _get_nc(with_debug)
    in_maps = _make_in_maps(
        inputs["x"], inputs["W"], inputs["b"], inputs["Wg"], inputs["bg"]
    )
    res = run_bass_kernel_spmd(
        nc, in_maps, core_ids=list(range(N_CORES)), **spmd_kwargs
    )
    out = np.concatenate(
        [res.results[c]["out"] for c in range(N_CORES)], axis=0
    ).reshape(B, S, O).astype(np.float32)
    return out, res


def kernel(x, W, b, Wg, bg):
    out, _ = run({"x": x, "W": W, "b": b, "Wg": Wg, "bg": bg})
    return out
